# revision 6
# baseline (speedup 1.0000x reference)
"""HaarMSELoss kernel for Trainium2 (8 NeuronCores, data-parallel).

Math: the 2x2 Haar transform used by the reference is (up to the 0.5
scaling) an orthogonal Hadamard transform, so for each 2x2 block
LL^2+LH^2+HL^2+HH^2 == a^2+b^2+c^2+d^2 of the block entries of
(input - target).  Hence

  loss = sum_bands mean((haar(x)-haar(y))^2)
       = sum((x-y)^2) / (B*C*(H/2)*(W/2))

i.e. a pure squared-difference reduction.  Each core reduces 1/8 of the
elements; the host sums the 8x128 per-partition partials (f64) and
divides.

Layout: per core the two chunks are interleaved host-side into one
[128, 2, FREE] array (row p = x-row p, y-row p) so each SBUF tile of
both operands arrives with a single dma_start.

Per-core DMA bandwidth is capped ~400 GB/s (HBM domain share / SBUF AXI
fabric) and a single HWDGE dma_start already fans out across all 16
SDMA engines, so one queue saturates it; the streaming phase is the
roofline.  Tile widths taper at the end (4096 x7, 2048, 1024, 512, 512)
so the serial sub+square tail after the last byte lands is minimal.

A dma_start's then_inc(sem, 16) is delivered as 16 independent +1
increments (one per SDMA engine as it finishes its slice), so a
cumulative wait sem >= 16*(t+1) can pass spuriously when engines skew
across tiles.  Each tile therefore gets its OWN semaphore and consumers
wait for exactly 16 on it.

Raw bass pipeline (explicit sems; every wait is a single-sem wait):
  SP  : dma loads (slot-recycled against ACT), final stats store
  DVE : d = x - y in place
  ACT : stats[:,t] = sum(d^2) via activation(Square, accum_out)
"""

import numpy as np

_B, _C, _H, _W = 4, 32, 512, 512
_TOTAL = _B * _C * _H * _W          # 33_554_432
_NCORES = 8
_PER_CORE = _TOTAL // _NCORES       # 4_194_304
_P = 128
_FREE = _PER_CORE // _P             # 32_768 f32 per partition per tensor
_WIDTHS = [4096] * 7 + [2048, 1024, 512, 512]   # sums to 32768
_NT = len(_WIDTHS)                  # 11 tiles
_OFFS = [sum(_WIDTHS[:i]) for i in range(_NT)]
_NBUF = 4
_WMAX = 4096
_DIVISOR = float(_TOTAL // 4)       # 8_388_608  (elements per subband)

_CACHE = {}


def _build_nc():
    from contextlib import ExitStack
    import concourse.bass as bass
    import concourse.mybir as mybir

    f32 = mybir.dt.float32
    nc = bass.Bass("TRN2", target_bir_lowering=False)
    xy = nc.dram_tensor("xy", [_P, 2, _FREE], f32, kind="ExternalInput")
    out = nc.dram_tensor("out", [_P, _NT], f32, kind="ExternalOutput")

    ctx = ExitStack()
    nc._ctx = ctx  # keep SBUF/semaphore handles alive for compile
    slots = [ctx.enter_context(nc.sbuf_tensor(f"slot{i}", [_P, 2, _WMAX], f32))
             for i in range(_NBUF)]
    stats = ctx.enter_context(nc.sbuf_tensor([_P, _NT], f32))
    zbias = ctx.enter_context(nc.sbuf_tensor([_P, 1], f32))
    tile_sems = [ctx.enter_context(nc.semaphore(f"tile_sem{t}"))
                 for t in range(_NT)]
    store_sem = ctx.enter_context(nc.semaphore("store_sem"))
    dve_sem = ctx.enter_context(nc.semaphore())
    act_sem = ctx.enter_context(nc.semaphore())
    block = ctx.enter_context(nc.Block())

    @block.sync
    def _(sync):
        for t in range(_NT):
            if t >= _NBUF:
                # slot free once ACT (last reader) finished tile t-NBUF
                sync.wait_ge(act_sem, t - _NBUF + 1)
            w, o = _WIDTHS[t], _OFFS[t]
            st = slots[t % _NBUF]
            sync.dma_start(
                out=st[:, :, :w], in_=xy[:, :, o:o + w]
            ).then_inc(tile_sems[t], 16)
        # act_sem increments fire on ACTIVATION_READ_ACCUMULATOR complete,
        # so stats is fully written before the store is generated; all
        # loads were consumed by DVE before then
        sync.wait_ge(act_sem, _NT)
        sync.dma_start(out=out[:], in_=stats[:]).then_inc(store_sem, 16)
        sync.wait_ge(store_sem, 16)  # store landed

    @block.vector
    def _(vector):
        vector.memset(zbias[:], 0.0).then_inc(dve_sem, 1)
        for t in range(_NT):
            w = _WIDTHS[t]
            st = slots[t % _NBUF]
            vector.wait_ge(tile_sems[t], 16)
            vector.tensor_sub(st[:, 0, :w], st[:, 0, :w], st[:, 1, :w]) \
                  .then_inc(dve_sem, 1)

    @block.scalar
    def _(scalar):
        for t in range(_NT):
            w = _WIDTHS[t]
            st = slots[t % _NBUF]
            scalar.wait_ge(dve_sem, t + 2)
            scalar.activation(
                st[:, 0, :w], st[:, 0, :w],
                mybir.ActivationFunctionType.Square,
                bias=zbias[:, 0:1], accum_out=stats[:, t:t + 1],
            ).then_inc(act_sem, 1)

    ctx.close()
    return nc


def _run(in_maps, trace=False):
    from concourse.bass_utils import run_bass_kernel_spmd

    if "nc" not in _CACHE:
        _CACHE["nc"] = _build_nc()
    return run_bass_kernel_spmd(
        _CACHE["nc"], in_maps, list(range(_NCORES)), trace=trace
    )


def _make_in_maps(input, target):
    xs = np.asarray(input, dtype=np.float32).reshape(_NCORES, _P, _FREE)
    ys = np.asarray(target, dtype=np.float32).reshape(_NCORES, _P, _FREE)
    maps = []
    for c in range(_NCORES):
        xy = np.empty((_P, 2, _FREE), dtype=np.float32)
        xy[:, 0, :] = xs[c]
        xy[:, 1, :] = ys[c]
        maps.append({"xy": xy})
    return maps


def _finish(results):
    total = 0.0
    for r in results:
        total += r["out"].astype(np.float64).sum()
    return np.array(total / _DIVISOR, dtype=np.float32)


def kernel(input, target):
    res = _run(_make_in_maps(input, target), trace=False)
    return _finish(res.results)


# revision 8
# speedup vs baseline: 4.9869x; 4.9869x over previous
"""HaarMSELoss kernel for Trainium2 (8 NeuronCores, data-parallel).

Math: the 2x2 Haar transform used by the reference is (up to the 0.5
scaling) an orthogonal Hadamard transform, so for each 2x2 block
LL^2+LH^2+HL^2+HH^2 == a^2+b^2+c^2+d^2 of the block entries of
(input - target).  Hence

  loss = sum_bands mean((haar(x)-haar(y))^2)
       = sum((x-y)^2) / (B*C*(H/2)*(W/2))

i.e. a pure squared-difference reduction.  Each core reduces 1/8 of the
elements; the host sums the 8x128 per-partition partials (f64) and
divides.

Layout: per core the two chunks are interleaved host-side into one
[128, 2, FREE] array (row p = x-row p, y-row p) so each SBUF tile of
both operands arrives with a single dma_start.

Per-core DMA bandwidth is capped ~400 GB/s (HBM domain share / SBUF AXI
fabric) and a single HWDGE dma_start already fans out across all 16
SDMA engines, so one queue saturates it; the streaming phase is the
roofline.  Tiles are 2048 wide with 10 slots so the slot-reuse gate
(dma issue waits on the square that last read the slot) references
compute ~9 tiles in the past and the queue never runs dry.  Widths
taper at the end (1024, 512, 512) so the serial sub+square tail after
the last byte lands is minimal.

A dma_start's then_inc(sem, 16) is delivered as 16 independent +1
increments (one per SDMA engine as it finishes its slice), so a
cumulative wait sem >= 16*(t+1) can pass spuriously when engines skew
across tiles.  Each tile therefore gets its OWN semaphore and consumers
wait for exactly 16 on it.

Raw bass pipeline (explicit sems; every wait is a single-sem wait):
  SP  : dma loads (slot-recycled against ACT), final stats store
  DVE : d = x - y in place
  ACT : stats[:,t] = sum(d^2) via activation(Square, accum_out)
"""

import numpy as np

_B, _C, _H, _W = 4, 32, 512, 512
_TOTAL = _B * _C * _H * _W          # 33_554_432
_NCORES = 8
_PER_CORE = _TOTAL // _NCORES       # 4_194_304
_P = 128
_FREE = _PER_CORE // _P             # 32_768 f32 per partition per tensor
_WIDTHS = [2048] * 15 + [1024, 512, 512]   # sums to 32768
_NT = len(_WIDTHS)                  # 18 tiles
_OFFS = [sum(_WIDTHS[:i]) for i in range(_NT)]
_NBUF = 10
_WMAX = 2048
_DIVISOR = float(_TOTAL // 4)       # 8_388_608  (elements per subband)

_CACHE = {}


def _build_nc():
    from contextlib import ExitStack
    import concourse.bass as bass
    import concourse.mybir as mybir

    f32 = mybir.dt.float32
    nc = bass.Bass("TRN2", target_bir_lowering=False)
    xy = nc.dram_tensor("xy", [_P, 2, _FREE], f32, kind="ExternalInput")
    out = nc.dram_tensor("out", [_P, _NT], f32, kind="ExternalOutput")

    ctx = ExitStack()
    nc._ctx = ctx  # keep SBUF/semaphore handles alive for compile
    slots = [ctx.enter_context(nc.sbuf_tensor(f"slot{i}", [_P, 2, _WMAX], f32))
             for i in range(_NBUF)]
    stats = ctx.enter_context(nc.sbuf_tensor([_P, _NT], f32))
    zbias = ctx.enter_context(nc.sbuf_tensor([_P, 1], f32))
    tile_sems = [ctx.enter_context(nc.semaphore(f"tile_sem{t}"))
                 for t in range(_NT)]
    store_sem = ctx.enter_context(nc.semaphore("store_sem"))
    dve_sem = ctx.enter_context(nc.semaphore())
    act_sem = ctx.enter_context(nc.semaphore())
    block = ctx.enter_context(nc.Block())

    @block.sync
    def _(sync):
        for t in range(_NT):
            if t >= _NBUF:
                # slot free once ACT (last reader) finished tile t-NBUF
                sync.wait_ge(act_sem, t - _NBUF + 1)
            w, o = _WIDTHS[t], _OFFS[t]
            st = slots[t % _NBUF]
            sync.dma_start(
                out=st[:, :, :w], in_=xy[:, :, o:o + w]
            ).then_inc(tile_sems[t], 16)
        # act_sem increments fire on ACTIVATION_READ_ACCUMULATOR complete,
        # so stats is fully written before the store is generated; all
        # loads were consumed by DVE before then
        sync.wait_ge(act_sem, _NT)
        sync.dma_start(out=out[:], in_=stats[:]).then_inc(store_sem, 16)
        sync.wait_ge(store_sem, 16)  # store landed

    @block.vector
    def _(vector):
        vector.memset(zbias[:], 0.0).then_inc(dve_sem, 1)
        for t in range(_NT):
            w = _WIDTHS[t]
            st = slots[t % _NBUF]
            vector.wait_ge(tile_sems[t], 16)
            vector.tensor_sub(st[:, 0, :w], st[:, 0, :w], st[:, 1, :w]) \
                  .then_inc(dve_sem, 1)

    @block.scalar
    def _(scalar):
        for t in range(_NT):
            w = _WIDTHS[t]
            st = slots[t % _NBUF]
            scalar.wait_ge(dve_sem, t + 2)
            scalar.activation(
                st[:, 0, :w], st[:, 0, :w],
                mybir.ActivationFunctionType.Square,
                bias=zbias[:, 0:1], accum_out=stats[:, t:t + 1],
            ).then_inc(act_sem, 1)

    ctx.close()
    return nc


def _run(in_maps, trace=False):
    from concourse.bass_utils import run_bass_kernel_spmd

    if "nc" not in _CACHE:
        _CACHE["nc"] = _build_nc()
    return run_bass_kernel_spmd(
        _CACHE["nc"], in_maps, list(range(_NCORES)), trace=trace
    )


def _make_in_maps(input, target):
    xs = np.asarray(input, dtype=np.float32).reshape(_NCORES, _P, _FREE)
    ys = np.asarray(target, dtype=np.float32).reshape(_NCORES, _P, _FREE)
    maps = []
    for c in range(_NCORES):
        xy = np.empty((_P, 2, _FREE), dtype=np.float32)
        xy[:, 0, :] = xs[c]
        xy[:, 1, :] = ys[c]
        maps.append({"xy": xy})
    return maps


def _finish(results):
    total = 0.0
    for r in results:
        total += r["out"].astype(np.float64).sum()
    return np.array(total / _DIVISOR, dtype=np.float32)


def kernel(input, target):
    res = _run(_make_in_maps(input, target), trace=False)
    return _finish(res.results)
